# revision 1
# baseline (speedup 1.0000x reference)
"""Trainium2 Bass kernel for the ConditionalPredicateHead GNN edge-MLP.

Per-edge computation (reference):
    out[e] = relu([emb[src[e]] | emb[tgt[e]] | u[batch[src[e]]]] @ W1 + b1) @ W2 + b2

Strategy (8 NeuronCores, edges data-parallel, 65536 edges/core):
  Host prep (layout only): int32 index conversion, per-core edge sort by src
  (so batch[src] is segment-contiguous), padding each graph segment to a
  32-edge boundary, p-major shuffling so indirect-DMA gathers land tile
  aligned, and inverse permutation of the outputs.

  Device per core:
    - indirect DMA gathers of fp32 embedding rows (512B descriptors) with
      cast-to-fp16 in flight,
    - PE transposes (fp16) of each 128-edge tile to get features onto
      partitions,
    - mm1: W1a/W1b stationary fp16 matmuls accumulating into PSUM (N=512),
    - the u-term + b1 folded into a per-partition bias applied by the
      ScalarE relu (bias column per 32-edge group, gathered on device from a
      u @ W1c + b1 table computed on device),
    - mm2: H^T (fp16) as stationary operand against W2, + b2, DMA out.
"""

import numpy as np
from contextlib import ExitStack

import concourse.bass as bass
import concourse.tile as tile
import concourse.mybir as mybir
from concourse.bass import IndirectOffsetOnAxis
from concourse.bass_utils import run_bass_kernel_spmd
from concourse.masks import make_identity

F32 = mybir.dt.float32
F16 = mybir.dt.float16
I32 = mybir.dt.int32

N_CORES = 8
E_FULL = 524288
N_NODES = 50000
HID = 128
GDIM = 8
NPRED = 9
NGRAPH = 64
IN_DIM = 2 * HID + GDIM

E_CORE = E_FULL // N_CORES    # 65536
CHUNK = 4096                  # edges gathered per indirect DMA
KPP = CHUNK // 128            # 32 edges per partition per chunk
BIAS_GRAN = 32                # edges per relu-bias group (graph-uniform)
T_PAD = 69632                 # padded edges per core (= 17 chunks)
ST_EDGES = 512                # edges per matmul supertile


def _split_excess_waits(nc, limit=1):
    """walrus CoreV3 codegen rejects instructions with more than `limit`
    semaphore waits; move extras onto injected same-engine nops placed
    right before the instruction (program order preserved per engine)."""
    n = 0
    for f in nc.m.functions:
        for b in f.blocks:
            out = []
            for inst in b.instructions:
                si = inst.sync_info
                waits = list(si.on_wait) if si is not None and si.on_wait else []
                if len(waits) > limit:
                    extra, keep = waits[:-limit], waits[-limit:]
                    for i in range(0, len(extra), limit):
                        nop = mybir.InstNoOp(
                            name=nc.get_next_instruction_name(),
                            ins=[], outs=[],
                            sync_info=mybir.SyncInfo(
                                on_wait=list(extra[i:i + limit]), on_update=[]),
                        )
                        nop.engine = inst.engine
                        nc.register_instruction(nop)
                        out.append(nop)
                        n += 1
                    si.on_wait = keep
                out.append(inst)
            b.instructions[:] = out
    return n


def build_nc(t_pad=T_PAD, gather_cast=True):
    """Build the per-core SPMD Bass program (identical across cores)."""
    nchunk = t_pad // CHUNK
    ngroup = t_pad // BIAS_GRAN
    ntpp = (ngroup + 127) // 128          # bias-gather rows per partition
    nst = CHUNK // ST_EDGES               # supertiles per chunk (8)

    nc = bass.Bass()
    emb = nc.dram_tensor("emb", [N_NODES, HID], F32, kind="ExternalInput")
    srcx = nc.dram_tensor("src", [nchunk, 128, KPP], I32, kind="ExternalInput")
    tgtx = nc.dram_tensor("tgt", [nchunk, 128, KPP], I32, kind="ExternalInput")
    gtx = nc.dram_tensor("gt", [128, ntpp], I32, kind="ExternalInput")
    u_x = nc.dram_tensor("u", [NGRAPH, GDIM], F32, kind="ExternalInput")
    w1x = nc.dram_tensor("W1", [IN_DIM, HID], F32, kind="ExternalInput")
    b1x = nc.dram_tensor("b1", [1, HID], F32, kind="ExternalInput")
    w2x = nc.dram_tensor("W2", [HID, NPRED], F32, kind="ExternalInput")
    b2x = nc.dram_tensor("b2", [1, NPRED], F32, kind="ExternalInput")
    outx = nc.dram_tensor("out_shuf", [nchunk, 128, KPP * NPRED], F32,
                          kind="ExternalOutput")
    uwb1_dram = nc.dram_tensor("uwb1_scratch", [NGRAPH, HID], F32)

    with tile.TileContext(nc) as tc, ExitStack() as ctx:
        const = ctx.enter_context(tc.tile_pool(name="const", bufs=1))

        # ---- constants / weights ----
        ident32 = const.tile([128, 128], F32)
        make_identity(nc, ident32[:])
        ident16 = const.tile([128, 128], F16)
        make_identity(nc, ident16[:])

        w1a16 = const.tile([128, 128], F16, tag="w1a")
        w1b16 = const.tile([128, 128], F16, tag="w1b")
        w216 = const.tile([128, NPRED], F16, tag="w2")
        b2bc = const.tile([128, KPP * NPRED], F32, tag="b2bc")
        bias_T = const.tile([128, ntpp * 128], F32, tag="biasT")

        with tc.tile_pool(name="setup", bufs=1) as sp, \
             tc.tile_pool(name="setup_ps", bufs=1, space="PSUM") as spp:
            # W1 halves -> fp16
            w1tmp = sp.tile([128, 128], F32, tag="w1tmp")
            nc.sync.dma_start(out=w1tmp[:], in_=w1x[0:128, :])
            nc.vector.tensor_copy(out=w1a16[:], in_=w1tmp[:])
            w1tmp2 = sp.tile([128, 128], F32, tag="w1tmp2")
            nc.sync.dma_start(out=w1tmp2[:], in_=w1x[128:256, :])
            nc.vector.tensor_copy(out=w1b16[:], in_=w1tmp2[:])
            # W2 -> fp16
            w2tmp = sp.tile([128, NPRED], F32, tag="w2tmp")
            nc.sync.dma_start(out=w2tmp[:], in_=w2x[:, :])
            nc.vector.tensor_copy(out=w216[:], in_=w2tmp[:])

            # UWb1 = u @ W1c + b1  via ones-augmented matmul
            rhs9 = sp.tile([9, 128], F32, tag="rhs9")
            nc.sync.dma_start(out=rhs9[0:8, :], in_=w1x[256:264, :])
            nc.sync.dma_start(out=rhs9[8:9, :], in_=b1x[:, :])
            u_t = sp.tile([NGRAPH, GDIM], F32, tag="u_t")
            nc.sync.dma_start(out=u_t[:], in_=u_x[:, :])
            ps_ut = spp.tile([GDIM, NGRAPH], F32, space="PSUM", tag="ps_ut")
            nc.tensor.transpose(out=ps_ut[:], in_=u_t[:],
                                identity=ident32[0:NGRAPH, 0:NGRAPH])
            lhs9 = sp.tile([9, NGRAPH], F32, tag="lhs9")
            nc.vector.memset(lhs9[:], 1.0)
            nc.vector.tensor_copy(out=lhs9[0:8, :], in_=ps_ut[:])
            ps_uw = spp.tile([NGRAPH, HID], F32, space="PSUM", tag="ps_uw")
            nc.tensor.matmul(out=ps_uw[:], lhsT=lhs9[:], rhs=rhs9[:],
                             start=True, stop=True)
            uw_sb = sp.tile([NGRAPH, HID], F32, tag="uw_sb")
            nc.vector.tensor_copy(out=uw_sb[:], in_=ps_uw[:])
            nc.sync.dma_start(out=uwb1_dram[:, :], in_=uw_sb[:])

            # b2 broadcast to all partitions, tiled KPP times along free dim
            b2row = sp.tile([1, KPP * NPRED], F32, tag="b2row")
            for j in range(KPP):
                nc.sync.dma_start(
                    out=b2row[:, j * NPRED:(j + 1) * NPRED],
                    in_=b2x[:, :])
            ones1 = sp.tile([1, 128], F32, tag="ones1")
            nc.vector.memset(ones1[:], 1.0)
            ps_b2 = spp.tile([128, KPP * NPRED], F32, space="PSUM", tag="ps_b2")
            nc.tensor.matmul(out=ps_b2[:], lhsT=ones1[:], rhs=b2row[:],
                             start=True, stop=True)
            nc.vector.tensor_copy(out=b2bc[:], in_=ps_b2[:])

            # barrier: uwb1_dram write must land before the bias gather
            tc.strict_bb_all_engine_barrier()

            # gather per-group bias rows and transpose into bias_T columns
            gt_t = sp.tile([128, ntpp], I32, tag="gt_t")
            nc.sync.dma_start(out=gt_t[:], in_=gtx[:, :])
            bias_rows = sp.tile([128, ntpp * HID], F32, tag="bias_rows")
            for i in range(ntpp):
                nc.gpsimd.indirect_dma_start(
                    out=bias_rows[:, i * HID:(i + 1) * HID], out_offset=None,
                    in_=uwb1_dram[:],
                    in_offset=IndirectOffsetOnAxis(ap=gt_t[:, i:i + 1], axis=0))
            for i in range(ntpp):
                ps_bt = spp.tile([128, 128], F32, space="PSUM", tag="ps_bt")
                nc.tensor.transpose(
                    out=ps_bt[:], in_=bias_rows[:, i * 128:(i + 1) * 128],
                    identity=ident32[:])
                nc.vector.tensor_copy(
                    out=bias_T[:, i * 128:(i + 1) * 128], in_=ps_bt[:])

        # ---- main loop ----
        with tc.tile_pool(name="idx", bufs=3) as idxp, \
             tc.tile_pool(name="gath", bufs=3) as gathp, \
             tc.tile_pool(name="work", bufs=2) as workp, \
             tc.tile_pool(name="outp", bufs=2) as outp, \
             tc.tile_pool(name="ps", bufs=2, space="PSUM") as psp:
            for c in range(nchunk):
                src_i = idxp.tile([128, KPP], I32, tag="src_i")
                nc.sync.dma_start(out=src_i[:], in_=srcx[c])
                tgt_i = idxp.tile([128, KPP], I32, tag="tgt_i")
                nc.sync.dma_start(out=tgt_i[:], in_=tgtx[c])

                gdt = F16 if gather_cast else F32
                gs = gathp.tile([128, CHUNK], gdt, tag="gs")
                gt_ = gathp.tile([128, CHUNK], gdt, tag="gt")
                for j in range(KPP):
                    nc.gpsimd.indirect_dma_start(
                        out=gs[:, j * HID:(j + 1) * HID], out_offset=None,
                        in_=emb[:],
                        in_offset=IndirectOffsetOnAxis(
                            ap=src_i[:, j:j + 1], axis=0))
                    nc.gpsimd.indirect_dma_start(
                        out=gt_[:, j * HID:(j + 1) * HID], out_offset=None,
                        in_=emb[:],
                        in_offset=IndirectOffsetOnAxis(
                            ap=tgt_i[:, j:j + 1], axis=0))
                if not gather_cast:
                    gs16 = gathp.tile([128, CHUNK], F16, tag="gs16")
                    nc.scalar.copy(out=gs16[:], in_=gs[:])
                    gt16 = gathp.tile([128, CHUNK], F16, tag="gt16")
                    nc.vector.tensor_copy(out=gt16[:], in_=gt_[:])
                    gs, gt_ = gs16, gt16

                out_sb = outp.tile([128, KPP * NPRED], F32, tag="out_sb")

                for st in range(nst):
                    ps_s = psp.tile([128, ST_EDGES], F16, space="PSUM", tag="ps_s")
                    ps_t = psp.tile([128, ST_EDGES], F16, space="PSUM", tag="ps_t")
                    for jj in range(ST_EDGES // 128):
                        j = st * (ST_EDGES // 128) + jj
                        nc.tensor.transpose(
                            out=ps_s[:, jj * 128:(jj + 1) * 128],
                            in_=gs[:, j * 128:(j + 1) * 128], identity=ident16[:])
                        nc.tensor.transpose(
                            out=ps_t[:, jj * 128:(jj + 1) * 128],
                            in_=gt_[:, j * 128:(j + 1) * 128], identity=ident16[:])
                    srcT = workp.tile([128, ST_EDGES], F16, tag="srcT")
                    nc.vector.tensor_copy(out=srcT[:], in_=ps_s[:])
                    tgtT = workp.tile([128, ST_EDGES], F16, tag="tgtT")
                    nc.vector.tensor_copy(out=tgtT[:], in_=ps_t[:])

                    ps_h = psp.tile([128, ST_EDGES], F32, space="PSUM", tag="ps_h")
                    nc.tensor.matmul(out=ps_h[:], lhsT=w1a16[:], rhs=srcT[:],
                                     start=True, stop=False)
                    nc.tensor.matmul(out=ps_h[:], lhsT=w1b16[:], rhs=tgtT[:],
                                     start=False, stop=True)

                    hT = workp.tile([128, ST_EDGES], F16, tag="hT")
                    ngr = ST_EDGES // BIAS_GRAN
                    for q in range(ngr):
                        gidx = (c * CHUNK + st * ST_EDGES) // BIAS_GRAN + q
                        nc.scalar.activation(
                            out=hT[:, q * BIAS_GRAN:(q + 1) * BIAS_GRAN],
                            in_=ps_h[:, q * BIAS_GRAN:(q + 1) * BIAS_GRAN],
                            func=mybir.ActivationFunctionType.Relu,
                            bias=bias_T[:, gidx:gidx + 1])

                    ps_o = psp.tile([128, (ST_EDGES // 128) * NPRED], F32,
                                    space="PSUM", tag="ps_o")
                    for jj in range(ST_EDGES // 128):
                        nc.tensor.matmul(
                            out=ps_o[:, jj * NPRED:(jj + 1) * NPRED],
                            lhsT=hT[:, jj * 128:(jj + 1) * 128], rhs=w216[:],
                            start=True, stop=True)
                    o0 = st * (ST_EDGES // 128) * NPRED
                    o1 = (st + 1) * (ST_EDGES // 128) * NPRED
                    nc.vector.tensor_tensor(
                        out=out_sb[:, o0:o1], in0=ps_o[:], in1=b2bc[:, o0:o1],
                        op=mybir.AluOpType.add)

                nc.sync.dma_start(out=outx[c], in_=out_sb[:])

    _split_excess_waits(nc, limit=1)
    return nc


# ---------------------------------------------------------------- host side

def prep_core(src, tgt, batch_np, t_pad=T_PAD):
    """Sort one core's edges by src, pad graph segments to BIAS_GRAN, shuffle
    p-major per chunk. Returns device input arrays + output unpermute info."""
    e_core = src.shape[0]
    nchunk = t_pad // CHUNK
    ngroup = t_pad // BIAS_GRAN
    ntpp = (ngroup + 127) // 128

    perm = np.argsort(src, kind="stable")
    src_s = src[perm]
    tgt_s = tgt[perm]
    g_s = batch_np[src_s]

    change = np.nonzero(np.diff(g_s))[0] + 1
    starts = np.concatenate([[0], change])
    ends = np.concatenate([change, [e_core]])

    src_pad = np.empty(t_pad, np.int32)
    tgt_pad = np.empty(t_pad, np.int32)
    g_pad = np.empty(t_pad, np.int32)
    padded_pos = np.empty(e_core, np.int64)
    pos = 0
    for s, e in zip(starts, ends):
        n = e - s
        src_pad[pos:pos + n] = src_s[s:e]
        tgt_pad[pos:pos + n] = tgt_s[s:e]
        g_pad[pos:pos + n] = g_s[s]
        padded_pos[s:e] = pos + np.arange(n)
        pos += n
        r = (-n) % BIAS_GRAN
        if r:
            src_pad[pos:pos + r] = src_s[e - 1]
            tgt_pad[pos:pos + r] = tgt_s[e - 1]
            g_pad[pos:pos + r] = g_s[s]
            pos += r
    assert pos <= t_pad, (pos, t_pad)
    src_pad[pos:] = src_s[-1]
    tgt_pad[pos:] = tgt_s[-1]
    g_pad[pos:] = g_s[-1]

    gtile = g_pad[::BIAS_GRAN].copy()          # (ngroup,)
    gt_full = np.zeros(ntpp * 128, np.int32)
    gt_full[:ngroup] = gtile
    # gt_shuf[p, i] = gtile[i*128 + p]
    gt_shuf = np.ascontiguousarray(gt_full.reshape(ntpp, 128).T)

    # shuf[c*CHUNK + p*KPP + j] = pad[c*CHUNK + j*128 + p]
    def shuffle(a):
        return np.ascontiguousarray(
            a.reshape(nchunk, KPP, 128).transpose(0, 2, 1)
        ).reshape(nchunk, 128, KPP)

    src_shuf = shuffle(src_pad)
    tgt_shuf = shuffle(tgt_pad)

    # shuffled flat position of each sorted real edge
    q = padded_pos
    cc, r = q // CHUNK, q % CHUNK
    jq, pq = r // 128, r % 128
    s_of_sorted = cc * CHUNK + pq * KPP + jq

    return dict(src=src_shuf, tgt=tgt_shuf, gt=gt_shuf,
                perm=perm, s_of_sorted=s_of_sorted)


_NC_CACHE = {}


def _get_nc(t_pad=T_PAD, gather_cast=True):
    key = (t_pad, gather_cast)
    if key not in _NC_CACHE:
        _NC_CACHE[key] = build_nc(t_pad=t_pad, gather_cast=gather_cast)
    return _NC_CACHE[key]


def make_in_maps(node_embeddings, edge_index, u, batch, W1, b1, W2, b2,
                 t_pad=T_PAD):
    emb = np.ascontiguousarray(np.asarray(node_embeddings, dtype=np.float32))
    ei = np.asarray(edge_index)
    src_all = ei[0].astype(np.int32)
    tgt_all = ei[1].astype(np.int32)
    batch_np = np.asarray(batch).astype(np.int32)
    u_np = np.ascontiguousarray(np.asarray(u, dtype=np.float32))
    W1_np = np.ascontiguousarray(np.asarray(W1, dtype=np.float32))
    b1_np = np.ascontiguousarray(np.asarray(b1, dtype=np.float32))
    W2_np = np.ascontiguousarray(np.asarray(W2, dtype=np.float32))
    b2_np = np.ascontiguousarray(np.asarray(b2, dtype=np.float32))

    in_maps, metas = [], []
    for c in range(N_CORES):
        sl = slice(c * E_CORE, (c + 1) * E_CORE)
        pc = prep_core(src_all[sl], tgt_all[sl], batch_np, t_pad=t_pad)
        in_maps.append({
            "emb": emb, "src": pc["src"], "tgt": pc["tgt"], "gt": pc["gt"],
            "u": u_np, "W1": W1_np, "b1": b1_np.reshape(1, HID),
        "W2": W2_np, "b2": b2_np.reshape(1, NPRED),
        })
        metas.append(pc)
    return in_maps, metas


def assemble_output(results, metas):
    out = np.empty((E_FULL, NPRED), np.float32)
    for c in range(N_CORES):
        o = np.asarray(results[c]["out_shuf"], dtype=np.float32)
        o = o.reshape(-1, NPRED)           # flat shuffled (t_pad, 9)
        pc = metas[c]
        core_out = np.empty((E_CORE, NPRED), np.float32)
        core_out[pc["perm"]] = o[pc["s_of_sorted"]]
        out[c * E_CORE:(c + 1) * E_CORE] = core_out
    return out


def kernel(node_embeddings, edge_index, u, batch, W1, b1, W2, b2):
    in_maps, metas = make_in_maps(node_embeddings, edge_index, u, batch,
                                  W1, b1, W2, b2)
    nc = _get_nc()
    res = run_bass_kernel_spmd(nc, in_maps, list(range(N_CORES)))
    return assemble_output(res.results, metas)



# revision 10
# speedup vs baseline: 1.1560x; 1.1560x over previous
"""Trainium2 Bass kernel for the ConditionalPredicateHead GNN edge-MLP.

Per-edge computation (reference):
    out[e] = relu([emb[src[e]] | emb[tgt[e]] | u[batch[src[e]]]] @ W1 + b1) @ W2 + b2

Strategy (8 NeuronCores, edges data-parallel, 65536 edges/core):
  Host prep (layout only): int32 index conversion, fp16 cast of the
  embedding table, per-core edge sort by src graph (so batch[src] is
  segment-contiguous), padding each graph segment to a 32-edge boundary,
  p-major shuffling so indirect-DMA gathers land tile aligned, and inverse
  permutation of the outputs.

  Device per core:
    - indirect DMA gathers of fp16 embedding rows (256B descriptors),
    - PE transposes (fp16) of each 128-edge tile to get features onto
      partitions,
    - mm1 into PSUM (N=512 supertile): a bias matmul (per-32-edge-group
      bias rows x one-hot expansion constant) + W1a/W1b stationary fp16
      matmuls. The bias rows are u @ W1c + b1 (computed on device once),
      gathered per group from DRAM,
    - full-width ScalarE relu PSUM -> fp16 SBUF,
    - mm2: hT (fp16) stationary against W2 accumulated into a per-chunk
      PSUM tile, + b2 via one DVE add, DMA out.
"""

import numpy as np
from contextlib import ExitStack

import concourse.bass as bass
import concourse.tile as tile
import concourse.mybir as mybir
from concourse.bass import IndirectOffsetOnAxis
from concourse.bass_utils import run_bass_kernel_spmd
from concourse.masks import make_identity

F32 = mybir.dt.float32
F16 = mybir.dt.float16
I32 = mybir.dt.int32

N_CORES = 8
E_FULL = 524288
N_NODES = 50000
HID = 128
GDIM = 8
NPRED = 9
NGRAPH = 64
IN_DIM = 2 * HID + GDIM

E_CORE = E_FULL // N_CORES    # 65536
CHUNK = 4096                  # edges gathered per chunk
KPP = CHUNK // 128            # 32 j-tiles per chunk
BIAS_GRAN = 32                # edges per bias group (graph-uniform)
T_PAD = 69632                 # padded edges per core (= 17 chunks)
ST_EDGES = 512                # edges per matmul supertile
NST = CHUNK // ST_EDGES       # supertiles per chunk (8)
GPC = CHUNK // BIAS_GRAN      # bias groups per chunk (128)
GPST = ST_EDGES // BIAS_GRAN  # bias groups per supertile (16)


def _split_excess_waits(nc, limit=1):
    """walrus CoreV3 codegen rejects instructions with more than `limit`
    semaphore waits; move extras onto injected same-engine nops placed
    right before the instruction (program order preserved per engine)."""
    n = 0
    for f in nc.m.functions:
        for b in f.blocks:
            out = []
            for inst in b.instructions:
                si = inst.sync_info
                waits = list(si.on_wait) if si is not None and si.on_wait else []
                if len(waits) > limit:
                    extra, keep = waits[:-limit], waits[-limit:]
                    for i in range(0, len(extra), limit):
                        nop = mybir.InstNoOp(
                            name=nc.get_next_instruction_name(),
                            ins=[], outs=[],
                            sync_info=mybir.SyncInfo(
                                on_wait=list(extra[i:i + limit]), on_update=[]),
                        )
                        nop.engine = inst.engine
                        nc.register_instruction(nop)
                        out.append(nop)
                        n += 1
                    si.on_wait = keep
                out.append(inst)
            b.instructions[:] = out
    return n


def build_nc(t_pad=T_PAD, gather_cast=True):
    """Build the per-core SPMD Bass program (identical across cores)."""
    nchunk = t_pad // CHUNK
    ngroup = t_pad // BIAS_GRAN
    ntpp = (ngroup + 127) // 128          # bias-gather rows per partition

    nc = bass.Bass()
    emb = nc.dram_tensor("emb", [N_NODES, HID], F16, kind="ExternalInput")
    srcx = nc.dram_tensor("src", [nchunk, 128, KPP], I32, kind="ExternalInput")
    tgtx = nc.dram_tensor("tgt", [nchunk, 128, KPP], I32, kind="ExternalInput")
    gtx = nc.dram_tensor("gt", [128, ntpp], I32, kind="ExternalInput")
    e128x = nc.dram_tensor("e128", [128, NST * ST_EDGES], F16,
                           kind="ExternalInput")
    u_x = nc.dram_tensor("u", [NGRAPH, GDIM], F32, kind="ExternalInput")
    w1x = nc.dram_tensor("W1", [IN_DIM, HID], F32, kind="ExternalInput")
    b1x = nc.dram_tensor("b1", [1, HID], F32, kind="ExternalInput")
    w2x = nc.dram_tensor("W2", [HID, NPRED], F32, kind="ExternalInput")
    b2x = nc.dram_tensor("b2", [1, NPRED], F32, kind="ExternalInput")
    outx = nc.dram_tensor("out_shuf", [nchunk, 128, KPP * NPRED], F32,
                          kind="ExternalOutput")
    uwb1_dram = nc.dram_tensor("uwb1_scratch", [NGRAPH, HID], F16)

    with tile.TileContext(nc) as tc, ExitStack() as ctx:
        const = ctx.enter_context(tc.tile_pool(name="const", bufs=1))

        # ---- constants / weights ----
        ident32 = const.tile([128, 128], F32)
        make_identity(nc, ident32[:])
        ident16 = const.tile([128, 128], F16)
        make_identity(nc, ident16[:])

        w1a16 = const.tile([128, 128], F16, tag="w1a")
        w1b16 = const.tile([128, 128], F16, tag="w1b")
        w216 = const.tile([128, NPRED], F16, tag="w2")
        b2bc = const.tile([128, KPP * NPRED], F32, tag="b2bc")
        e128 = const.tile([128, NST * ST_EDGES], F16, tag="e128")
        bias_rows = const.tile([128, ntpp * HID], F16, tag="bias_rows")

        nc.sync.dma_start(out=e128[:], in_=e128x[:, :])

        with tc.tile_pool(name="setup", bufs=1) as sp, \
             tc.tile_pool(name="setup_ps", bufs=1, space="PSUM") as spp:
            # W1 halves -> fp16
            w1tmp = sp.tile([128, 128], F32, tag="w1tmp")
            nc.sync.dma_start(out=w1tmp[:], in_=w1x[0:128, :])
            nc.vector.tensor_copy(out=w1a16[:], in_=w1tmp[:])
            w1tmp2 = sp.tile([128, 128], F32, tag="w1tmp2")
            nc.sync.dma_start(out=w1tmp2[:], in_=w1x[128:256, :])
            nc.vector.tensor_copy(out=w1b16[:], in_=w1tmp2[:])
            # W2 -> fp16
            w2tmp = sp.tile([128, NPRED], F32, tag="w2tmp")
            nc.sync.dma_start(out=w2tmp[:], in_=w2x[:, :])
            nc.vector.tensor_copy(out=w216[:], in_=w2tmp[:])

            # UWb1 = u @ W1c + b1  via ones-augmented matmul
            rhs9 = sp.tile([9, 128], F32, tag="rhs9")
            nc.sync.dma_start(out=rhs9[0:8, :], in_=w1x[256:264, :])
            nc.sync.dma_start(out=rhs9[8:9, :], in_=b1x[:, :])
            u_t = sp.tile([NGRAPH, GDIM], F32, tag="u_t")
            nc.sync.dma_start(out=u_t[:], in_=u_x[:, :])
            ps_ut = spp.tile([GDIM, NGRAPH], F32, space="PSUM", tag="ps_ut")
            nc.tensor.transpose(out=ps_ut[:], in_=u_t[:],
                                identity=ident32[0:NGRAPH, 0:NGRAPH])
            lhs9 = sp.tile([9, NGRAPH], F32, tag="lhs9")
            nc.vector.memset(lhs9[:], 1.0)
            nc.vector.tensor_copy(out=lhs9[0:8, :], in_=ps_ut[:])
            ps_uw = spp.tile([NGRAPH, HID], F32, space="PSUM", tag="ps_uw")
            nc.tensor.matmul(out=ps_uw[:], lhsT=lhs9[:], rhs=rhs9[:],
                             start=True, stop=True)
            uw_sb = sp.tile([NGRAPH, HID], F16, tag="uw_sb")
            nc.vector.tensor_copy(out=uw_sb[:], in_=ps_uw[:])
            nc.sync.dma_start(out=uwb1_dram[:, :], in_=uw_sb[:])

            # b2 broadcast to all partitions, tiled KPP times along free dim
            b2row = sp.tile([1, KPP * NPRED], F32, tag="b2row")
            for j in range(KPP):
                nc.sync.dma_start(
                    out=b2row[:, j * NPRED:(j + 1) * NPRED],
                    in_=b2x[:, :])
            ones1 = sp.tile([1, 128], F32, tag="ones1")
            nc.vector.memset(ones1[:], 1.0)
            ps_b2 = spp.tile([128, KPP * NPRED], F32, space="PSUM", tag="ps_b2")
            nc.tensor.matmul(out=ps_b2[:], lhsT=ones1[:], rhs=b2row[:],
                             start=True, stop=True)
            nc.vector.tensor_copy(out=b2bc[:], in_=ps_b2[:])

            # barrier: uwb1_dram write must land before the bias gather
            tc.strict_bb_all_engine_barrier()

            # gather per-group bias rows: bias_rows[p, i*HID:(i+1)*HID] =
            # UWb1[graph of group i*128+p]
            gt_t = sp.tile([128, ntpp], I32, tag="gt_t")
            nc.sync.dma_start(out=gt_t[:], in_=gtx[:, :])
            for i in range(ntpp):
                nc.gpsimd.indirect_dma_start(
                    out=bias_rows[:, i * HID:(i + 1) * HID], out_offset=None,
                    in_=uwb1_dram[:],
                    in_offset=IndirectOffsetOnAxis(ap=gt_t[:, i:i + 1], axis=0))

        # ---- main loop ----
        with tc.tile_pool(name="idx", bufs=3) as idxp, \
             tc.tile_pool(name="gath", bufs=3) as gathp, \
             tc.tile_pool(name="work", bufs=2) as workp, \
             tc.tile_pool(name="outp", bufs=2) as outp, \
             tc.tile_pool(name="ps", bufs=2, space="PSUM") as psp, \
             tc.tile_pool(name="pso", bufs=2, space="PSUM") as psop:
            for c in range(nchunk):
                src_i = idxp.tile([128, KPP], I32, tag="src_i")
                nc.sync.dma_start(out=src_i[:], in_=srcx[c])
                tgt_i = idxp.tile([128, KPP], I32, tag="tgt_i")
                nc.sync.dma_start(out=tgt_i[:], in_=tgtx[c])

                gs = gathp.tile([128, CHUNK], F16, tag="gs")
                gt_ = gathp.tile([128, CHUNK], F16, tag="gt")
                for j in range(KPP):
                    nc.gpsimd.indirect_dma_start(
                        out=gs[:, j * HID:(j + 1) * HID], out_offset=None,
                        in_=emb[:],
                        in_offset=IndirectOffsetOnAxis(
                            ap=src_i[:, j:j + 1], axis=0))
                    nc.gpsimd.indirect_dma_start(
                        out=gt_[:, j * HID:(j + 1) * HID], out_offset=None,
                        in_=emb[:],
                        in_offset=IndirectOffsetOnAxis(
                            ap=tgt_i[:, j:j + 1], axis=0))

                ps_o = psop.tile([128, KPP * NPRED], F32, space="PSUM",
                                 tag="ps_o")
                out_sb = outp.tile([128, KPP * NPRED], F32, tag="out_sb")

                for st in range(NST):
                    ps_s = psp.tile([128, ST_EDGES], F16, space="PSUM",
                                    tag="ps_s")
                    ps_t = psp.tile([128, ST_EDGES], F16, space="PSUM",
                                    tag="ps_t")
                    for jj in range(ST_EDGES // 128):
                        j = st * (ST_EDGES // 128) + jj
                        nc.tensor.transpose(
                            out=ps_s[:, jj * 128:(jj + 1) * 128],
                            in_=gs[:, j * 128:(j + 1) * 128], identity=ident16[:])
                        nc.tensor.transpose(
                            out=ps_t[:, jj * 128:(jj + 1) * 128],
                            in_=gt_[:, j * 128:(j + 1) * 128], identity=ident16[:])
                    srcT = workp.tile([128, ST_EDGES], F16, tag="srcT")
                    nc.vector.tensor_copy(out=srcT[:], in_=ps_s[:])
                    tgtT = workp.tile([128, ST_EDGES], F16, tag="tgtT")
                    nc.vector.tensor_copy(out=tgtT[:], in_=ps_t[:])

                    # mm1: bias (one-hot expansion) + W1a/W1b into PSUM
                    ps_h = psp.tile([128, ST_EDGES], F32, space="PSUM",
                                    tag="ps_h")
                    nc.tensor.matmul(
                        out=ps_h[:],
                        lhsT=bias_rows[:, c * HID:(c + 1) * HID],
                        rhs=e128[:, st * ST_EDGES:(st + 1) * ST_EDGES],
                        start=True, stop=False)
                    nc.tensor.matmul(out=ps_h[:], lhsT=w1a16[:], rhs=srcT[:],
                                     start=False, stop=False)
                    nc.tensor.matmul(out=ps_h[:], lhsT=w1b16[:], rhs=tgtT[:],
                                     start=False, stop=True)

                    hT = workp.tile([128, ST_EDGES], F16, tag="hT")
                    nc.scalar.activation(
                        out=hT[:], in_=ps_h[:],
                        func=mybir.ActivationFunctionType.Relu)

                    for jj in range(ST_EDGES // 128):
                        j = st * (ST_EDGES // 128) + jj
                        nc.tensor.matmul(
                            out=ps_o[:, j * NPRED:(j + 1) * NPRED],
                            lhsT=hT[:, jj * 128:(jj + 1) * 128], rhs=w216[:],
                            start=True, stop=True)

                nc.vector.tensor_tensor(
                    out=out_sb[:], in0=ps_o[:], in1=b2bc[:],
                    op=mybir.AluOpType.add)
                nc.sync.dma_start(out=outx[c], in_=out_sb[:])

    _split_excess_waits(nc, limit=1)
    return nc


# ---------------------------------------------------------------- host side

def prep_core(src, tgt, batch_np, t_pad=T_PAD):
    """Sort one core's edges by src graph, pad graph segments to BIAS_GRAN,
    shuffle p-major per chunk. Returns device input arrays + unpermute info."""
    e_core = src.shape[0]
    nchunk = t_pad // CHUNK
    ngroup = t_pad // BIAS_GRAN
    ntpp = (ngroup + 127) // 128

    perm = np.argsort(src, kind="stable")
    src_s = src[perm]
    tgt_s = tgt[perm]
    g_s = batch_np[src_s]

    change = np.nonzero(np.diff(g_s))[0] + 1
    starts = np.concatenate([[0], change])
    ends = np.concatenate([change, [e_core]])

    src_pad = np.empty(t_pad, np.int32)
    tgt_pad = np.empty(t_pad, np.int32)
    g_pad = np.empty(t_pad, np.int32)
    padded_pos = np.empty(e_core, np.int64)
    pos = 0
    for s, e in zip(starts, ends):
        n = e - s
        src_pad[pos:pos + n] = src_s[s:e]
        tgt_pad[pos:pos + n] = tgt_s[s:e]
        g_pad[pos:pos + n] = g_s[s]
        padded_pos[s:e] = pos + np.arange(n)
        pos += n
        r = (-n) % BIAS_GRAN
        if r:
            src_pad[pos:pos + r] = src_s[e - 1]
            tgt_pad[pos:pos + r] = tgt_s[e - 1]
            g_pad[pos:pos + r] = g_s[s]
            pos += r
    assert pos <= t_pad, (pos, t_pad)
    src_pad[pos:] = src_s[-1]
    tgt_pad[pos:] = tgt_s[-1]
    g_pad[pos:] = g_s[-1]

    gtile = g_pad[::BIAS_GRAN].copy()          # (ngroup,)
    gt_full = np.zeros(ntpp * 128, np.int32)
    gt_full[:ngroup] = gtile
    # gt_shuf[p, i] = gtile[i*128 + p]
    gt_shuf = np.ascontiguousarray(gt_full.reshape(ntpp, 128).T)

    # shuf[c*CHUNK + p*KPP + j] = pad[c*CHUNK + j*128 + p]
    def shuffle(a):
        return np.ascontiguousarray(
            a.reshape(nchunk, KPP, 128).transpose(0, 2, 1)
        ).reshape(nchunk, 128, KPP)

    src_shuf = shuffle(src_pad)
    tgt_shuf = shuffle(tgt_pad)

    # shuffled flat position of each sorted real edge
    q = padded_pos
    cc, r = q // CHUNK, q % CHUNK
    jq, pq = r // 128, r % 128
    s_of_sorted = cc * CHUNK + pq * KPP + jq

    return dict(src=src_shuf, tgt=tgt_shuf, gt=gt_shuf,
                perm=perm, s_of_sorted=s_of_sorted)


def make_e128():
    """e128[st][g, e] = 1 iff group-of-e-within-supertile == g - st*GPST.
    Shipped as [128, NST*ST_EDGES] matching the SBUF tile layout."""
    e = np.zeros((NST, 128, ST_EDGES), np.float16)
    for st in range(NST):
        for q in range(GPST):
            e[st, st * GPST + q, q * BIAS_GRAN:(q + 1) * BIAS_GRAN] = 1.0
    return np.ascontiguousarray(
        e.transpose(1, 0, 2).reshape(128, NST * ST_EDGES))


_NC_CACHE = {}


def _get_nc(t_pad=T_PAD, gather_cast=True):
    key = (t_pad, gather_cast)
    if key not in _NC_CACHE:
        _NC_CACHE[key] = build_nc(t_pad=t_pad, gather_cast=gather_cast)
    return _NC_CACHE[key]


def make_in_maps(node_embeddings, edge_index, u, batch, W1, b1, W2, b2,
                 t_pad=T_PAD):
    emb16 = np.ascontiguousarray(
        np.asarray(node_embeddings, dtype=np.float32).astype(np.float16))
    ei = np.asarray(edge_index)
    src_all = ei[0].astype(np.int32)
    tgt_all = ei[1].astype(np.int32)
    batch_np = np.asarray(batch).astype(np.int32)
    u_np = np.ascontiguousarray(np.asarray(u, dtype=np.float32))
    W1_np = np.ascontiguousarray(np.asarray(W1, dtype=np.float32))
    b1_np = np.ascontiguousarray(np.asarray(b1, dtype=np.float32))
    W2_np = np.ascontiguousarray(np.asarray(W2, dtype=np.float32))
    b2_np = np.ascontiguousarray(np.asarray(b2, dtype=np.float32))
    e128 = make_e128()

    in_maps, metas = [], []
    for c in range(N_CORES):
        sl = slice(c * E_CORE, (c + 1) * E_CORE)
        pc = prep_core(src_all[sl], tgt_all[sl], batch_np, t_pad=t_pad)
        in_maps.append({
            "emb": emb16, "src": pc["src"], "tgt": pc["tgt"], "gt": pc["gt"],
            "e128": e128, "u": u_np, "W1": W1_np,
            "b1": b1_np.reshape(1, HID),
            "W2": W2_np, "b2": b2_np.reshape(1, NPRED),
        })
        metas.append(pc)
    return in_maps, metas


def assemble_output(results, metas):
    out = np.empty((E_FULL, NPRED), np.float32)
    for c in range(N_CORES):
        o = np.asarray(results[c]["out_shuf"], dtype=np.float32)
        o = o.reshape(-1, NPRED)           # flat shuffled (t_pad, 9)
        pc = metas[c]
        core_out = np.empty((E_CORE, NPRED), np.float32)
        core_out[pc["perm"]] = o[pc["s_of_sorted"]]
        out[c * E_CORE:(c + 1) * E_CORE] = core_out
    return out


def kernel(node_embeddings, edge_index, u, batch, W1, b1, W2, b2):
    in_maps, metas = make_in_maps(node_embeddings, edge_index, u, batch,
                                  W1, b1, W2, b2)
    nc = _get_nc()
    res = run_bass_kernel_spmd(nc, in_maps, list(range(N_CORES)))
    return assemble_output(res.results, metas)


# revision 21
# speedup vs baseline: 2.0025x; 1.7322x over previous
"""Trainium2 Bass kernel for the ConditionalPredicateHead GNN edge-MLP.

Per-edge computation (reference):
    out[e] = relu([emb[src[e]] | emb[tgt[e]] | u[batch[src[e]]]] @ W1 + b1) @ W2 + b2

Strategy (8 NeuronCores, edges data-parallel, 65536 edges/core):
  Host prep (layout only): int32 index conversion, fp16 cast of the
  embedding table, per-core edge sort by src graph (so batch[src] is
  segment-contiguous), padding each graph segment to a 32-edge boundary,
  p-major shuffling so indirect-DMA gathers land tile aligned, and inverse
  permutation of the outputs.

  Device per core:
    - indirect DMA gathers of fp16 embedding rows (256B descriptors),
    - PE transposes (fp16) of each 128-edge tile to get features onto
      partitions,
    - mm1 into PSUM (N=512 supertile): a bias matmul (per-32-edge-group
      bias rows x one-hot expansion constant) + W1a/W1b stationary fp16
      matmuls. The bias rows are u @ W1c + b1 (computed on device once),
      gathered per group from DRAM,
    - full-width ScalarE relu PSUM -> fp16 SBUF,
    - mm2: hT (fp16) stationary against W2 accumulated into a per-chunk
      PSUM tile, + b2 via one DVE add, DMA out.
"""

import numpy as np
from contextlib import ExitStack

import concourse.bass as bass
import concourse.tile as tile
import concourse.mybir as mybir
from concourse.bass import IndirectOffsetOnAxis
from concourse.bass_utils import run_bass_kernel_spmd
from concourse.masks import make_identity

F32 = mybir.dt.float32
F16 = mybir.dt.float16
I32 = mybir.dt.int32

N_CORES = 8
E_FULL = 524288
N_NODES = 50000
HID = 128
GDIM = 8
NPRED = 9
NGRAPH = 64
IN_DIM = 2 * HID + GDIM

E_CORE = E_FULL // N_CORES    # 65536
CHUNK = 4096                  # edges gathered per chunk
KPP = CHUNK // 128            # 32 j-tiles per chunk
BIAS_GRAN = 32                # edges per bias group (graph-uniform)
T_PAD = 69632                 # padded edges per core (= 17 chunks)
ST_EDGES = 512                # edges per matmul supertile
NST = CHUNK // ST_EDGES       # supertiles per chunk (8)
GPC = CHUNK // BIAS_GRAN      # bias groups per chunk (128)
GPST = ST_EDGES // BIAS_GRAN  # bias groups per supertile (16)


def _split_excess_waits(nc, limit=1):
    """walrus CoreV3 codegen rejects instructions with more than `limit`
    semaphore waits; move extras onto injected same-engine nops placed
    right before the instruction (program order preserved per engine)."""
    n = 0
    for f in nc.m.functions:
        for b in f.blocks:
            out = []
            for inst in b.instructions:
                si = inst.sync_info
                waits = list(si.on_wait) if si is not None and si.on_wait else []
                if len(waits) > limit:
                    extra, keep = waits[:-limit], waits[-limit:]
                    for i in range(0, len(extra), limit):
                        nop = mybir.InstNoOp(
                            name=nc.get_next_instruction_name(),
                            ins=[], outs=[],
                            sync_info=mybir.SyncInfo(
                                on_wait=list(extra[i:i + limit]), on_update=[]),
                        )
                        nop.engine = inst.engine
                        nc.register_instruction(nop)
                        out.append(nop)
                        n += 1
                    si.on_wait = keep
                out.append(inst)
            b.instructions[:] = out
    return n


def build_nc(t_pad=T_PAD, gather_cast=True, reps=1, only_gathers=False,
             no_gathers=False, gath_bufs=5):
    """Build the per-core SPMD Bass program (identical across cores).
    reps>1 repeats the main loop (for marginal-cost timing only)."""
    nchunk = t_pad // CHUNK
    ngroup = t_pad // BIAS_GRAN
    ntpp = (ngroup + 127) // 128          # bias-gather rows per partition

    nc = bass.Bass()
    emb = nc.dram_tensor("emb", [N_NODES, HID], F16, kind="ExternalInput")
    srcx = nc.dram_tensor("src", [nchunk, 128, KPP], I32, kind="ExternalInput")
    tgtx = nc.dram_tensor("tgt", [nchunk, 128, KPP], I32, kind="ExternalInput")
    gtx = nc.dram_tensor("gt", [128, ntpp], I32, kind="ExternalInput")
    e128x = nc.dram_tensor("e128", [128, NST * ST_EDGES], F16,
                           kind="ExternalInput")
    u_x = nc.dram_tensor("u", [NGRAPH, GDIM], F32, kind="ExternalInput")
    w1x = nc.dram_tensor("W1", [IN_DIM, HID], F32, kind="ExternalInput")
    b1x = nc.dram_tensor("b1", [1, HID], F32, kind="ExternalInput")
    w2x = nc.dram_tensor("W2", [HID, NPRED], F32, kind="ExternalInput")
    b2x = nc.dram_tensor("b2", [1, NPRED], F32, kind="ExternalInput")
    outx = nc.dram_tensor("out_shuf", [nchunk, 128, KPP * NPRED], F32,
                          kind="ExternalOutput")
    uwb1_dram = nc.dram_tensor("uwb1_scratch", [NGRAPH, HID], F16)

    with tile.TileContext(nc) as tc, ExitStack() as ctx:
        const = ctx.enter_context(tc.tile_pool(name="const", bufs=1))

        # ---- constants / weights ----
        ident32 = const.tile([128, 128], F32)
        make_identity(nc, ident32[:])
        ident16 = const.tile([128, 128], F16)
        make_identity(nc, ident16[:])

        w1a16 = const.tile([128, 128], F16, tag="w1a")
        w1b16 = const.tile([128, 128], F16, tag="w1b")
        w216 = const.tile([128, NPRED], F16, tag="w2")
        b2bc = const.tile([128, KPP * NPRED], F32, tag="b2bc")
        e128 = const.tile([128, NST * ST_EDGES], F16, tag="e128")
        bias_rows = const.tile([128, ntpp * HID], F16, tag="bias_rows")

        nc.sync.dma_start(out=e128[:], in_=e128x[:, :])

        with tc.tile_pool(name="setup", bufs=1) as sp, \
             tc.tile_pool(name="setup_ps", bufs=1, space="PSUM") as spp:
            # W1 halves -> fp16
            w1tmp = sp.tile([128, 128], F32, tag="w1tmp")
            nc.sync.dma_start(out=w1tmp[:], in_=w1x[0:128, :])
            nc.vector.tensor_copy(out=w1a16[:], in_=w1tmp[:])
            w1tmp2 = sp.tile([128, 128], F32, tag="w1tmp2")
            nc.sync.dma_start(out=w1tmp2[:], in_=w1x[128:256, :])
            nc.vector.tensor_copy(out=w1b16[:], in_=w1tmp2[:])
            # W2 -> fp16
            w2tmp = sp.tile([128, NPRED], F32, tag="w2tmp")
            nc.sync.dma_start(out=w2tmp[:], in_=w2x[:, :])
            nc.vector.tensor_copy(out=w216[:], in_=w2tmp[:])

            # UWb1 = u @ W1c + b1  via ones-augmented matmul
            rhs9 = sp.tile([9, 128], F32, tag="rhs9")
            nc.sync.dma_start(out=rhs9[0:8, :], in_=w1x[256:264, :])
            nc.sync.dma_start(out=rhs9[8:9, :], in_=b1x[:, :])
            u_t = sp.tile([NGRAPH, GDIM], F32, tag="u_t")
            nc.sync.dma_start(out=u_t[:], in_=u_x[:, :])
            ps_ut = spp.tile([GDIM, NGRAPH], F32, space="PSUM", tag="ps_ut")
            nc.tensor.transpose(out=ps_ut[:], in_=u_t[:],
                                identity=ident32[0:NGRAPH, 0:NGRAPH])
            lhs9 = sp.tile([9, NGRAPH], F32, tag="lhs9")
            nc.vector.memset(lhs9[:], 1.0)
            nc.vector.tensor_copy(out=lhs9[0:8, :], in_=ps_ut[:])
            ps_uw = spp.tile([NGRAPH, HID], F32, space="PSUM", tag="ps_uw")
            nc.tensor.matmul(out=ps_uw[:], lhsT=lhs9[:], rhs=rhs9[:],
                             start=True, stop=True)
            uw_sb = sp.tile([NGRAPH, HID], F16, tag="uw_sb")
            nc.vector.tensor_copy(out=uw_sb[:], in_=ps_uw[:])
            nc.sync.dma_start(out=uwb1_dram[:, :], in_=uw_sb[:])

            # b2 broadcast to all partitions, tiled KPP times along free dim
            b2row = sp.tile([1, KPP * NPRED], F32, tag="b2row")
            for j in range(KPP):
                nc.sync.dma_start(
                    out=b2row[:, j * NPRED:(j + 1) * NPRED],
                    in_=b2x[:, :])
            ones1 = sp.tile([1, 128], F32, tag="ones1")
            nc.vector.memset(ones1[:], 1.0)
            ps_b2 = spp.tile([128, KPP * NPRED], F32, space="PSUM", tag="ps_b2")
            nc.tensor.matmul(out=ps_b2[:], lhsT=ones1[:], rhs=b2row[:],
                             start=True, stop=True)
            nc.vector.tensor_copy(out=b2bc[:], in_=ps_b2[:])

            # barrier: uwb1_dram write must land before the bias gather
            tc.strict_bb_all_engine_barrier()

            # gather per-group bias rows: bias_rows[p, i*HID:(i+1)*HID] =
            # UWb1[graph of group i*128+p]
            gt_t = sp.tile([128, ntpp], I32, tag="gt_t")
            nc.sync.dma_start(out=gt_t[:], in_=gtx[:, :])
            for i in range(ntpp):
                nc.gpsimd.indirect_dma_start(
                    out=bias_rows[:, i * HID:(i + 1) * HID], out_offset=None,
                    in_=uwb1_dram[:],
                    in_offset=IndirectOffsetOnAxis(ap=gt_t[:, i:i + 1], axis=0))

        # ---- main loop ----
        with tc.tile_pool(name="idx", bufs=3) as idxp, \
             tc.tile_pool(name="gath", bufs=gath_bufs) as gathp, \
             tc.tile_pool(name="work", bufs=2) as workp, \
             tc.tile_pool(name="outp", bufs=2) as outp, \
             tc.tile_pool(name="ps", bufs=2, space="PSUM") as psp, \
             tc.tile_pool(name="pso", bufs=2, space="PSUM") as psop:
            for c in [c for _ in range(reps) for c in range(nchunk)]:
                src_i = idxp.tile([128, KPP], I32, tag="src_i")
                nc.sync.dma_start(out=src_i[:], in_=srcx[c])
                tgt_i = idxp.tile([128, KPP], I32, tag="tgt_i")
                nc.sync.dma_start(out=tgt_i[:], in_=tgtx[c])

                gs = gathp.tile([128, CHUNK], F16, tag="gs")
                gt_ = gathp.tile([128, CHUNK], F16, tag="gt")
                for j in ([] if no_gathers else range(KPP)):
                    nc.gpsimd.indirect_dma_start(
                        out=gs[:, j * HID:(j + 1) * HID], out_offset=None,
                        in_=emb[:],
                        in_offset=IndirectOffsetOnAxis(
                            ap=src_i[:, j:j + 1], axis=0))
                    nc.gpsimd.indirect_dma_start(
                        out=gt_[:, j * HID:(j + 1) * HID], out_offset=None,
                        in_=emb[:],
                        in_offset=IndirectOffsetOnAxis(
                            ap=tgt_i[:, j:j + 1], axis=0))

                if only_gathers:
                    # consume the gather tiles cheaply so pools rotate
                    tok = outp.tile([128, 16], F32, tag="tok")
                    nc.vector.tensor_copy(out=tok[:], in_=gs[:, 0:16])
                    nc.vector.tensor_copy(out=tok[:], in_=gt_[:, 0:16])
                    continue

                ps_o = psop.tile([128, KPP * NPRED], F32, space="PSUM",
                                 tag="ps_o")
                out_sb = outp.tile([128, KPP * NPRED], F32, tag="out_sb")

                for st in range(NST):
                    ps_s = psp.tile([128, ST_EDGES], F16, space="PSUM",
                                    tag="ps_s")
                    ps_t = psp.tile([128, ST_EDGES], F16, space="PSUM",
                                    tag="ps_t")
                    for jj in range(ST_EDGES // 128):
                        j = st * (ST_EDGES // 128) + jj
                        nc.tensor.transpose(
                            out=ps_s[:, jj * 128:(jj + 1) * 128],
                            in_=gs[:, j * 128:(j + 1) * 128], identity=ident16[:])
                        nc.tensor.transpose(
                            out=ps_t[:, jj * 128:(jj + 1) * 128],
                            in_=gt_[:, j * 128:(j + 1) * 128], identity=ident16[:])
                    srcT = workp.tile([128, ST_EDGES], F16, tag="srcT")
                    nc.vector.tensor_copy(out=srcT[:], in_=ps_s[:])
                    tgtT = workp.tile([128, ST_EDGES], F16, tag="tgtT")
                    nc.vector.tensor_copy(out=tgtT[:], in_=ps_t[:])

                    # mm1: bias (one-hot expansion) + W1a/W1b into PSUM
                    ps_h = psp.tile([128, ST_EDGES], F32, space="PSUM",
                                    tag="ps_h")
                    nc.tensor.matmul(
                        out=ps_h[:],
                        lhsT=bias_rows[:, c * HID:(c + 1) * HID],
                        rhs=e128[:, st * ST_EDGES:(st + 1) * ST_EDGES],
                        start=True, stop=False)
                    nc.tensor.matmul(out=ps_h[:], lhsT=w1a16[:], rhs=srcT[:],
                                     start=False, stop=False)
                    nc.tensor.matmul(out=ps_h[:], lhsT=w1b16[:], rhs=tgtT[:],
                                     start=False, stop=True)

                    hT = workp.tile([128, ST_EDGES], F16, tag="hT")
                    nc.scalar.activation(
                        out=hT[:], in_=ps_h[:],
                        func=mybir.ActivationFunctionType.Relu)

                    for jj in range(ST_EDGES // 128):
                        j = st * (ST_EDGES // 128) + jj
                        nc.tensor.matmul(
                            out=ps_o[:, j * NPRED:(j + 1) * NPRED],
                            lhsT=hT[:, jj * 128:(jj + 1) * 128], rhs=w216[:],
                            start=True, stop=True)

                nc.vector.tensor_tensor(
                    out=out_sb[:], in0=ps_o[:], in1=b2bc[:],
                    op=mybir.AluOpType.add)
                # out-write on the Activation queue (has slack), keeping the
                # sync queue free for index loads that gate the gathers
                nc.scalar.dma_start(out=outx[c], in_=out_sb[:])

    _split_excess_waits(nc, limit=1)
    return nc


# ---------------------------------------------------------------- host side

def prep_core(src, tgt, batch_np, t_pad=T_PAD):
    """Sort one core's edges by src graph, pad graph segments to BIAS_GRAN,
    shuffle p-major per chunk. Returns device input arrays + unpermute info."""
    e_core = src.shape[0]
    nchunk = t_pad // CHUNK
    ngroup = t_pad // BIAS_GRAN
    ntpp = (ngroup + 127) // 128

    perm = np.argsort(src, kind="stable")
    src_s = src[perm]
    tgt_s = tgt[perm]
    g_s = batch_np[src_s]

    change = np.nonzero(np.diff(g_s))[0] + 1
    starts = np.concatenate([[0], change])
    ends = np.concatenate([change, [e_core]])

    src_pad = np.empty(t_pad, np.int32)
    tgt_pad = np.empty(t_pad, np.int32)
    g_pad = np.empty(t_pad, np.int32)
    padded_pos = np.empty(e_core, np.int64)
    pos = 0
    for s, e in zip(starts, ends):
        n = e - s
        src_pad[pos:pos + n] = src_s[s:e]
        tgt_pad[pos:pos + n] = tgt_s[s:e]
        g_pad[pos:pos + n] = g_s[s]
        padded_pos[s:e] = pos + np.arange(n)
        pos += n
        r = (-n) % BIAS_GRAN
        if r:
            src_pad[pos:pos + r] = src_s[e - 1]
            tgt_pad[pos:pos + r] = tgt_s[e - 1]
            g_pad[pos:pos + r] = g_s[s]
            pos += r
    assert pos <= t_pad, (pos, t_pad)
    src_pad[pos:] = src_s[-1]
    tgt_pad[pos:] = tgt_s[-1]
    g_pad[pos:] = g_s[-1]

    gtile = g_pad[::BIAS_GRAN].copy()          # (ngroup,)
    gt_full = np.zeros(ntpp * 128, np.int32)
    gt_full[:ngroup] = gtile
    # gt_shuf[p, i] = gtile[i*128 + p]
    gt_shuf = np.ascontiguousarray(gt_full.reshape(ntpp, 128).T)

    # shuf[c*CHUNK + p*KPP + j] = pad[c*CHUNK + j*128 + p]
    def shuffle(a):
        return np.ascontiguousarray(
            a.reshape(nchunk, KPP, 128).transpose(0, 2, 1)
        ).reshape(nchunk, 128, KPP)

    src_shuf = shuffle(src_pad)
    tgt_shuf = shuffle(tgt_pad)

    # shuffled flat position of each sorted real edge
    q = padded_pos
    cc, r = q // CHUNK, q % CHUNK
    jq, pq = r // 128, r % 128
    s_of_sorted = cc * CHUNK + pq * KPP + jq

    return dict(src=src_shuf, tgt=tgt_shuf, gt=gt_shuf,
                perm=perm, s_of_sorted=s_of_sorted)


def make_e128():
    """e128[st][g, e] = 1 iff group-of-e-within-supertile == g - st*GPST.
    Shipped as [128, NST*ST_EDGES] matching the SBUF tile layout."""
    e = np.zeros((NST, 128, ST_EDGES), np.float16)
    for st in range(NST):
        for q in range(GPST):
            e[st, st * GPST + q, q * BIAS_GRAN:(q + 1) * BIAS_GRAN] = 1.0
    return np.ascontiguousarray(
        e.transpose(1, 0, 2).reshape(128, NST * ST_EDGES))


_NC_CACHE = {}


def _get_nc(t_pad=T_PAD, gather_cast=True):
    key = (t_pad, gather_cast)
    if key not in _NC_CACHE:
        _NC_CACHE[key] = build_nc(t_pad=t_pad, gather_cast=gather_cast)
    return _NC_CACHE[key]


def make_in_maps(node_embeddings, edge_index, u, batch, W1, b1, W2, b2,
                 t_pad=T_PAD):
    emb16 = np.ascontiguousarray(
        np.asarray(node_embeddings, dtype=np.float32).astype(np.float16))
    ei = np.asarray(edge_index)
    src_all = ei[0].astype(np.int32)
    tgt_all = ei[1].astype(np.int32)
    batch_np = np.asarray(batch).astype(np.int32)
    u_np = np.ascontiguousarray(np.asarray(u, dtype=np.float32))
    W1_np = np.ascontiguousarray(np.asarray(W1, dtype=np.float32))
    b1_np = np.ascontiguousarray(np.asarray(b1, dtype=np.float32))
    W2_np = np.ascontiguousarray(np.asarray(W2, dtype=np.float32))
    b2_np = np.ascontiguousarray(np.asarray(b2, dtype=np.float32))
    e128 = make_e128()

    in_maps, metas = [], []
    for c in range(N_CORES):
        sl = slice(c * E_CORE, (c + 1) * E_CORE)
        pc = prep_core(src_all[sl], tgt_all[sl], batch_np, t_pad=t_pad)
        in_maps.append({
            "emb": emb16, "src": pc["src"], "tgt": pc["tgt"], "gt": pc["gt"],
            "e128": e128, "u": u_np, "W1": W1_np,
            "b1": b1_np.reshape(1, HID),
            "W2": W2_np, "b2": b2_np.reshape(1, NPRED),
        })
        metas.append(pc)
    return in_maps, metas


def assemble_output(results, metas):
    out = np.empty((E_FULL, NPRED), np.float32)
    for c in range(N_CORES):
        o = np.asarray(results[c]["out_shuf"], dtype=np.float32)
        o = o.reshape(-1, NPRED)           # flat shuffled (t_pad, 9)
        pc = metas[c]
        core_out = np.empty((E_CORE, NPRED), np.float32)
        core_out[pc["perm"]] = o[pc["s_of_sorted"]]
        out[c * E_CORE:(c + 1) * E_CORE] = core_out
    return out


def kernel(node_embeddings, edge_index, u, batch, W1, b1, W2, b2):
    in_maps, metas = make_in_maps(node_embeddings, edge_index, u, batch,
                                  W1, b1, W2, b2)
    nc = _get_nc()
    res = run_bass_kernel_spmd(nc, in_maps, list(range(N_CORES)))
    return assemble_output(res.results, metas)


# revision 22
# speedup vs baseline: 2.8371x; 1.4168x over previous
"""Trainium2 Bass kernel for the ConditionalPredicateHead GNN edge-MLP.

Per-edge computation (reference):
    out[e] = relu([emb[src[e]] | emb[tgt[e]] | u[batch[src[e]]]] @ W1 + b1) @ W2 + b2

Strategy (8 NeuronCores, edges data-parallel, 65536 edges/core):
  Host prep (layout only): int32 index conversion, fp16 cast of the
  embedding table, per-core edge sort by src graph (so batch[src] is
  segment-contiguous), padding each graph segment to a 32-edge boundary,
  p-major shuffling so indirect-DMA gathers land tile aligned, and inverse
  permutation of the outputs.

  Device per core:
    - indirect DMA gathers of fp16 embedding rows (256B descriptors),
    - PE transposes (fp16) of each 128-edge tile to get features onto
      partitions,
    - mm1 into PSUM (N=512 supertile): a bias matmul (per-32-edge-group
      bias rows x one-hot expansion constant) + W1a/W1b stationary fp16
      matmuls. The bias rows are u @ W1c + b1 (computed on device once),
      gathered per group from DRAM,
    - full-width ScalarE relu PSUM -> fp16 SBUF,
    - mm2: hT (fp16) stationary against W2 accumulated into a per-chunk
      PSUM tile, + b2 via one DVE add, DMA out.
"""

import numpy as np
from contextlib import ExitStack

import concourse.bass as bass
import concourse.tile as tile
import concourse.mybir as mybir
from concourse.bass import IndirectOffsetOnAxis
from concourse.bass_utils import run_bass_kernel_spmd
from concourse.masks import make_identity

F32 = mybir.dt.float32
F16 = mybir.dt.float16
I32 = mybir.dt.int32

N_CORES = 8
E_FULL = 524288
N_NODES = 50000
HID = 128
GDIM = 8
NPRED = 9
NGRAPH = 64
IN_DIM = 2 * HID + GDIM

E_CORE = E_FULL // N_CORES    # 65536
CHUNK = 4096                  # edges gathered per chunk
KPP = CHUNK // 128            # 32 j-tiles per chunk
BIAS_GRAN = 32                # edges per bias group (graph-uniform)
T_PAD = 69632                 # padded edges per core (= 17 chunks)
ST_EDGES = 512                # edges per matmul supertile
NST = CHUNK // ST_EDGES       # supertiles per chunk (8)
GPC = CHUNK // BIAS_GRAN      # bias groups per chunk (128)
GPST = ST_EDGES // BIAS_GRAN  # bias groups per supertile (16)


def _split_excess_waits(nc, limit=1):
    """walrus CoreV3 codegen rejects instructions with more than `limit`
    semaphore waits; move extras onto injected same-engine nops placed
    right before the instruction (program order preserved per engine)."""
    n = 0
    for f in nc.m.functions:
        for b in f.blocks:
            out = []
            for inst in b.instructions:
                si = inst.sync_info
                waits = list(si.on_wait) if si is not None and si.on_wait else []
                if len(waits) > limit:
                    extra, keep = waits[:-limit], waits[-limit:]
                    for i in range(0, len(extra), limit):
                        nop = mybir.InstNoOp(
                            name=nc.get_next_instruction_name(),
                            ins=[], outs=[],
                            sync_info=mybir.SyncInfo(
                                on_wait=list(extra[i:i + limit]), on_update=[]),
                        )
                        nop.engine = inst.engine
                        nc.register_instruction(nop)
                        out.append(nop)
                        n += 1
                    si.on_wait = keep
                out.append(inst)
            b.instructions[:] = out
    return n


def build_nc(t_pad=T_PAD, gather_cast=True, reps=1, only_gathers=False,
             no_gathers=False, gath_bufs=5):
    """Build the per-core SPMD Bass program (identical across cores).
    reps>1 repeats the main loop (for marginal-cost timing only)."""
    nchunk = t_pad // CHUNK
    ngroup = t_pad // BIAS_GRAN
    ntpp = (ngroup + 127) // 128          # bias-gather rows per partition

    nc = bass.Bass()
    emb = nc.dram_tensor("emb", [N_NODES, HID], F16, kind="ExternalInput")
    srcx = nc.dram_tensor("src", [nchunk, 128, KPP], I32, kind="ExternalInput")
    tgtx = nc.dram_tensor("tgt", [nchunk, 128, KPP], I32, kind="ExternalInput")
    gtx = nc.dram_tensor("gt", [128, ntpp], I32, kind="ExternalInput")
    e128x = nc.dram_tensor("e128", [128, NST * ST_EDGES], F16,
                           kind="ExternalInput")
    u_x = nc.dram_tensor("u", [NGRAPH, GDIM], F32, kind="ExternalInput")
    w1x = nc.dram_tensor("W1", [IN_DIM, HID], F32, kind="ExternalInput")
    b1x = nc.dram_tensor("b1", [1, HID], F32, kind="ExternalInput")
    w2x = nc.dram_tensor("W2", [HID, NPRED], F32, kind="ExternalInput")
    b2x = nc.dram_tensor("b2", [1, NPRED], F32, kind="ExternalInput")
    outx = nc.dram_tensor("out_shuf", [nchunk, 128, KPP * NPRED], F32,
                          kind="ExternalOutput")
    uwb1_dram = nc.dram_tensor("uwb1_scratch", [NGRAPH, HID], F16)

    with tile.TileContext(nc) as tc, ExitStack() as ctx:
        const = ctx.enter_context(tc.tile_pool(name="const", bufs=1))

        # ---- constants / weights ----
        ident32 = const.tile([128, 128], F32)
        make_identity(nc, ident32[:])
        ident16 = const.tile([128, 128], F16)
        make_identity(nc, ident16[:])

        w1a16 = const.tile([128, 128], F16, tag="w1a")
        w1b16 = const.tile([128, 128], F16, tag="w1b")
        w216 = const.tile([128, NPRED], F16, tag="w2")
        b2bc = const.tile([128, KPP * NPRED], F32, tag="b2bc")
        e128 = const.tile([128, NST * ST_EDGES], F16, tag="e128")
        bias_rows = const.tile([128, ntpp * HID], F16, tag="bias_rows")

        nc.sync.dma_start(out=e128[:], in_=e128x[:, :])

        with tc.tile_pool(name="setup", bufs=1) as sp, \
             tc.tile_pool(name="setup_ps", bufs=1, space="PSUM") as spp:
            # W1 halves -> fp16
            w1tmp = sp.tile([128, 128], F32, tag="w1tmp")
            nc.sync.dma_start(out=w1tmp[:], in_=w1x[0:128, :])
            nc.vector.tensor_copy(out=w1a16[:], in_=w1tmp[:])
            w1tmp2 = sp.tile([128, 128], F32, tag="w1tmp2")
            nc.sync.dma_start(out=w1tmp2[:], in_=w1x[128:256, :])
            nc.vector.tensor_copy(out=w1b16[:], in_=w1tmp2[:])
            # W2 -> fp16
            w2tmp = sp.tile([128, NPRED], F32, tag="w2tmp")
            nc.sync.dma_start(out=w2tmp[:], in_=w2x[:, :])
            nc.vector.tensor_copy(out=w216[:], in_=w2tmp[:])

            # UWb1 = u @ W1c + b1  via ones-augmented matmul
            rhs9 = sp.tile([9, 128], F32, tag="rhs9")
            nc.sync.dma_start(out=rhs9[0:8, :], in_=w1x[256:264, :])
            nc.sync.dma_start(out=rhs9[8:9, :], in_=b1x[:, :])
            u_t = sp.tile([NGRAPH, GDIM], F32, tag="u_t")
            nc.sync.dma_start(out=u_t[:], in_=u_x[:, :])
            ps_ut = spp.tile([GDIM, NGRAPH], F32, space="PSUM", tag="ps_ut")
            nc.tensor.transpose(out=ps_ut[:], in_=u_t[:],
                                identity=ident32[0:NGRAPH, 0:NGRAPH])
            lhs9 = sp.tile([9, NGRAPH], F32, tag="lhs9")
            nc.vector.memset(lhs9[:], 1.0)
            nc.vector.tensor_copy(out=lhs9[0:8, :], in_=ps_ut[:])
            ps_uw = spp.tile([NGRAPH, HID], F32, space="PSUM", tag="ps_uw")
            nc.tensor.matmul(out=ps_uw[:], lhsT=lhs9[:], rhs=rhs9[:],
                             start=True, stop=True)
            uw_sb = sp.tile([NGRAPH, HID], F16, tag="uw_sb")
            nc.vector.tensor_copy(out=uw_sb[:], in_=ps_uw[:])
            nc.sync.dma_start(out=uwb1_dram[:, :], in_=uw_sb[:])

            # b2 broadcast to all partitions, tiled KPP times along free dim
            b2row = sp.tile([1, KPP * NPRED], F32, tag="b2row")
            for j in range(KPP):
                nc.sync.dma_start(
                    out=b2row[:, j * NPRED:(j + 1) * NPRED],
                    in_=b2x[:, :])
            ones1 = sp.tile([1, 128], F32, tag="ones1")
            nc.vector.memset(ones1[:], 1.0)
            ps_b2 = spp.tile([128, KPP * NPRED], F32, space="PSUM", tag="ps_b2")
            nc.tensor.matmul(out=ps_b2[:], lhsT=ones1[:], rhs=b2row[:],
                             start=True, stop=True)
            nc.vector.tensor_copy(out=b2bc[:], in_=ps_b2[:])

            # barrier: uwb1_dram write must land before the bias gather
            tc.strict_bb_all_engine_barrier()

            # gather per-group bias rows: bias_rows[p, i*HID:(i+1)*HID] =
            # UWb1[graph of group i*128+p]
            gt_t = sp.tile([128, ntpp], I32, tag="gt_t")
            nc.sync.dma_start(out=gt_t[:], in_=gtx[:, :])
            for i in range(ntpp):
                nc.gpsimd.indirect_dma_start(
                    out=bias_rows[:, i * HID:(i + 1) * HID], out_offset=None,
                    in_=uwb1_dram[:],
                    in_offset=IndirectOffsetOnAxis(ap=gt_t[:, i:i + 1], axis=0))

        # ---- main loop ----
        with tc.tile_pool(name="idx", bufs=6) as idxp, \
             tc.tile_pool(name="gath", bufs=gath_bufs) as gathp, \
             tc.tile_pool(name="work", bufs=2) as workp, \
             tc.tile_pool(name="outp", bufs=2) as outp, \
             tc.tile_pool(name="ps", bufs=2, space="PSUM") as psp, \
             tc.tile_pool(name="pso", bufs=2, space="PSUM") as psop:
            for c in [c for _ in range(reps) for c in range(nchunk)]:
                src_i = idxp.tile([128, KPP], I32, tag="src_i")
                nc.sync.dma_start(out=src_i[:], in_=srcx[c])
                tgt_i = idxp.tile([128, KPP], I32, tag="tgt_i")
                nc.sync.dma_start(out=tgt_i[:], in_=tgtx[c])

                gs = gathp.tile([128, CHUNK], F16, tag="gs")
                gt_ = gathp.tile([128, CHUNK], F16, tag="gt")
                for j in ([] if no_gathers else range(KPP)):
                    nc.gpsimd.indirect_dma_start(
                        out=gs[:, j * HID:(j + 1) * HID], out_offset=None,
                        in_=emb[:],
                        in_offset=IndirectOffsetOnAxis(
                            ap=src_i[:, j:j + 1], axis=0))
                    nc.gpsimd.indirect_dma_start(
                        out=gt_[:, j * HID:(j + 1) * HID], out_offset=None,
                        in_=emb[:],
                        in_offset=IndirectOffsetOnAxis(
                            ap=tgt_i[:, j:j + 1], axis=0))

                if only_gathers:
                    # consume the gather tiles cheaply so pools rotate
                    tok = outp.tile([128, 16], F32, tag="tok")
                    nc.vector.tensor_copy(out=tok[:], in_=gs[:, 0:16])
                    nc.vector.tensor_copy(out=tok[:], in_=gt_[:, 0:16])
                    continue

                ps_o = psop.tile([128, KPP * NPRED], F32, space="PSUM",
                                 tag="ps_o")
                out_sb = outp.tile([128, KPP * NPRED], F32, tag="out_sb")

                for st in range(NST):
                    ps_s = psp.tile([128, ST_EDGES], F16, space="PSUM",
                                    tag="ps_s")
                    ps_t = psp.tile([128, ST_EDGES], F16, space="PSUM",
                                    tag="ps_t")
                    for jj in range(ST_EDGES // 128):
                        j = st * (ST_EDGES // 128) + jj
                        nc.tensor.transpose(
                            out=ps_s[:, jj * 128:(jj + 1) * 128],
                            in_=gs[:, j * 128:(j + 1) * 128], identity=ident16[:])
                        nc.tensor.transpose(
                            out=ps_t[:, jj * 128:(jj + 1) * 128],
                            in_=gt_[:, j * 128:(j + 1) * 128], identity=ident16[:])
                    srcT = workp.tile([128, ST_EDGES], F16, tag="srcT")
                    nc.vector.tensor_copy(out=srcT[:], in_=ps_s[:])
                    tgtT = workp.tile([128, ST_EDGES], F16, tag="tgtT")
                    nc.vector.tensor_copy(out=tgtT[:], in_=ps_t[:])

                    # mm1: bias (one-hot expansion) + W1a/W1b into PSUM
                    ps_h = psp.tile([128, ST_EDGES], F32, space="PSUM",
                                    tag="ps_h")
                    nc.tensor.matmul(
                        out=ps_h[:],
                        lhsT=bias_rows[:, c * HID:(c + 1) * HID],
                        rhs=e128[:, st * ST_EDGES:(st + 1) * ST_EDGES],
                        start=True, stop=False)
                    nc.tensor.matmul(out=ps_h[:], lhsT=w1a16[:], rhs=srcT[:],
                                     start=False, stop=False)
                    nc.tensor.matmul(out=ps_h[:], lhsT=w1b16[:], rhs=tgtT[:],
                                     start=False, stop=True)

                    hT = workp.tile([128, ST_EDGES], F16, tag="hT")
                    nc.scalar.activation(
                        out=hT[:], in_=ps_h[:],
                        func=mybir.ActivationFunctionType.Relu)

                    for jj in range(ST_EDGES // 128):
                        j = st * (ST_EDGES // 128) + jj
                        nc.tensor.matmul(
                            out=ps_o[:, j * NPRED:(j + 1) * NPRED],
                            lhsT=hT[:, jj * 128:(jj + 1) * 128], rhs=w216[:],
                            start=True, stop=True)

                nc.vector.tensor_tensor(
                    out=out_sb[:], in0=ps_o[:], in1=b2bc[:],
                    op=mybir.AluOpType.add)
                # out-write on the Activation queue (has slack), keeping the
                # sync queue free for index loads that gate the gathers
                nc.scalar.dma_start(out=outx[c], in_=out_sb[:])

    _split_excess_waits(nc, limit=1)
    return nc


# ---------------------------------------------------------------- host side

def prep_core(src, tgt, batch_np, t_pad=T_PAD):
    """Sort one core's edges by src graph, pad graph segments to BIAS_GRAN,
    shuffle p-major per chunk. Returns device input arrays + unpermute info."""
    e_core = src.shape[0]
    nchunk = t_pad // CHUNK
    ngroup = t_pad // BIAS_GRAN
    ntpp = (ngroup + 127) // 128

    perm = np.argsort(src, kind="stable")
    src_s = src[perm]
    tgt_s = tgt[perm]
    g_s = batch_np[src_s]

    change = np.nonzero(np.diff(g_s))[0] + 1
    starts = np.concatenate([[0], change])
    ends = np.concatenate([change, [e_core]])

    src_pad = np.empty(t_pad, np.int32)
    tgt_pad = np.empty(t_pad, np.int32)
    g_pad = np.empty(t_pad, np.int32)
    padded_pos = np.empty(e_core, np.int64)
    pos = 0
    for s, e in zip(starts, ends):
        n = e - s
        src_pad[pos:pos + n] = src_s[s:e]
        tgt_pad[pos:pos + n] = tgt_s[s:e]
        g_pad[pos:pos + n] = g_s[s]
        padded_pos[s:e] = pos + np.arange(n)
        pos += n
        r = (-n) % BIAS_GRAN
        if r:
            src_pad[pos:pos + r] = src_s[e - 1]
            tgt_pad[pos:pos + r] = tgt_s[e - 1]
            g_pad[pos:pos + r] = g_s[s]
            pos += r
    assert pos <= t_pad, (pos, t_pad)
    src_pad[pos:] = src_s[-1]
    tgt_pad[pos:] = tgt_s[-1]
    g_pad[pos:] = g_s[-1]

    gtile = g_pad[::BIAS_GRAN].copy()          # (ngroup,)
    gt_full = np.zeros(ntpp * 128, np.int32)
    gt_full[:ngroup] = gtile
    # gt_shuf[p, i] = gtile[i*128 + p]
    gt_shuf = np.ascontiguousarray(gt_full.reshape(ntpp, 128).T)

    # shuf[c*CHUNK + p*KPP + j] = pad[c*CHUNK + j*128 + p]
    def shuffle(a):
        return np.ascontiguousarray(
            a.reshape(nchunk, KPP, 128).transpose(0, 2, 1)
        ).reshape(nchunk, 128, KPP)

    src_shuf = shuffle(src_pad)
    tgt_shuf = shuffle(tgt_pad)

    # shuffled flat position of each sorted real edge
    q = padded_pos
    cc, r = q // CHUNK, q % CHUNK
    jq, pq = r // 128, r % 128
    s_of_sorted = cc * CHUNK + pq * KPP + jq

    return dict(src=src_shuf, tgt=tgt_shuf, gt=gt_shuf,
                perm=perm, s_of_sorted=s_of_sorted)


def make_e128():
    """e128[st][g, e] = 1 iff group-of-e-within-supertile == g - st*GPST.
    Shipped as [128, NST*ST_EDGES] matching the SBUF tile layout."""
    e = np.zeros((NST, 128, ST_EDGES), np.float16)
    for st in range(NST):
        for q in range(GPST):
            e[st, st * GPST + q, q * BIAS_GRAN:(q + 1) * BIAS_GRAN] = 1.0
    return np.ascontiguousarray(
        e.transpose(1, 0, 2).reshape(128, NST * ST_EDGES))


_NC_CACHE = {}


def _get_nc(t_pad=T_PAD, gather_cast=True):
    key = (t_pad, gather_cast)
    if key not in _NC_CACHE:
        _NC_CACHE[key] = build_nc(t_pad=t_pad, gather_cast=gather_cast)
    return _NC_CACHE[key]


def make_in_maps(node_embeddings, edge_index, u, batch, W1, b1, W2, b2,
                 t_pad=T_PAD):
    emb16 = np.ascontiguousarray(
        np.asarray(node_embeddings, dtype=np.float32).astype(np.float16))
    ei = np.asarray(edge_index)
    src_all = ei[0].astype(np.int32)
    tgt_all = ei[1].astype(np.int32)
    batch_np = np.asarray(batch).astype(np.int32)
    u_np = np.ascontiguousarray(np.asarray(u, dtype=np.float32))
    W1_np = np.ascontiguousarray(np.asarray(W1, dtype=np.float32))
    b1_np = np.ascontiguousarray(np.asarray(b1, dtype=np.float32))
    W2_np = np.ascontiguousarray(np.asarray(W2, dtype=np.float32))
    b2_np = np.ascontiguousarray(np.asarray(b2, dtype=np.float32))
    e128 = make_e128()

    in_maps, metas = [], []
    for c in range(N_CORES):
        sl = slice(c * E_CORE, (c + 1) * E_CORE)
        pc = prep_core(src_all[sl], tgt_all[sl], batch_np, t_pad=t_pad)
        in_maps.append({
            "emb": emb16, "src": pc["src"], "tgt": pc["tgt"], "gt": pc["gt"],
            "e128": e128, "u": u_np, "W1": W1_np,
            "b1": b1_np.reshape(1, HID),
            "W2": W2_np, "b2": b2_np.reshape(1, NPRED),
        })
        metas.append(pc)
    return in_maps, metas


def assemble_output(results, metas):
    out = np.empty((E_FULL, NPRED), np.float32)
    for c in range(N_CORES):
        o = np.asarray(results[c]["out_shuf"], dtype=np.float32)
        o = o.reshape(-1, NPRED)           # flat shuffled (t_pad, 9)
        pc = metas[c]
        core_out = np.empty((E_CORE, NPRED), np.float32)
        core_out[pc["perm"]] = o[pc["s_of_sorted"]]
        out[c * E_CORE:(c + 1) * E_CORE] = core_out
    return out


def kernel(node_embeddings, edge_index, u, batch, W1, b1, W2, b2):
    in_maps, metas = make_in_maps(node_embeddings, edge_index, u, batch,
                                  W1, b1, W2, b2)
    nc = _get_nc()
    res = run_bass_kernel_spmd(nc, in_maps, list(range(N_CORES)))
    return assemble_output(res.results, metas)
